# revision 12
# baseline (speedup 1.0000x reference)
"""Trainium2 Bass kernel for the CrossLayer problem.

Math: reference computes, per row x (length D), with cur_0 = x:
    cur_{i+1} = sum(cur_i) * (w_i ⊙ x) + b_i + x        (i = 0..L-1)
Only the scalar s_i = sum(cur_i) couples elements, so with
    X   = sum(x)                  (per row)
    W_i = x · w_i                 (per row, i = 0..L-2)
    c_i = sum(b_i)
the recursion collapses to scalars:
    S_0 = X;  S_{i+1} = S_i * W_i + c_i + X
and the output is a single elementwise pass:
    out = S_{L-1} * (w_{L-1} ⊙ x) + b_{L-1} + x

Kernel layout (per core, pure data parallel over batch):
  - rows on partitions, 16 tiles of (128, 1024) f32, processed in pairs
  - PE transposes each tile chunk (128x128); ACT copies PSUM→SBUF into a
    paired buffer, then the tensor engine computes [X, W0, W1, W2] =
    Wpk^T @ x^T with N=256 moving (both tiles of the pair at once)
  - small PE transpose puts the dots row-major; ACT runs the scalar
    recursion (activation Identity with per-partition scale/bias)
  - final output on DVE: tensor_mul (w3 ⊙ x) + fused scalar_tensor_tensor
    (S3 * w3x + x); the general-b path adds b3 with one more pass
"""

import os
import numpy as np

B, D, L = 16384, 1024, 4
N_CORES = 8
RPC = B // N_CORES          # rows per core
P = 128                     # partitions
N_TILES = RPC // P          # 16
N_PAIRS = N_TILES // 2      # 8
N_CHUNKS = D // P           # 8

_built = {}


def _build_nc(b_zero: bool):
    import concourse.bass as bass
    import concourse.bacc as bacc
    import concourse.mybir as mybir
    from concourse import tile

    f32 = mybir.dt.float32
    Alu = mybir.AluOpType
    Act = mybir.ActivationFunctionType

    # Bacc (not raw Bass): its compile() legalizes semaphore waits — TRN2
    # matmuls encode at most one sync wait (walrus S3_LW struct).
    nc = bacc.Bacc(
        "TRN2", target_bir_lowering=False, debug=False, num_devices=N_CORES
    )
    x_d = nc.dram_tensor("x", [RPC, D], f32, kind="ExternalInput")
    wpk_d = nc.dram_tensor("wpk", [P, N_CHUNKS * 4], f32, kind="ExternalInput")
    w3bc_d = nc.dram_tensor("w3bc", [P, D], f32, kind="ExternalInput")
    ident_d = nc.dram_tensor("ident", [P, P], f32, kind="ExternalInput")
    if not b_zero:
        cvec_d = nc.dram_tensor("cvec", [P, 4], f32, kind="ExternalInput")
        b3bc_d = nc.dram_tensor("b3bc", [P, D], f32, kind="ExternalInput")
    out_d = nc.dram_tensor("out", [RPC, D], f32, kind="ExternalOutput")

    with tile.TileContext(nc) as tc:
        with (
            tc.tile_pool(name="consts", bufs=1) as consts,
            tc.tile_pool(name="xin", bufs=5) as xin_pool,
            tc.tile_pool(name="mid", bufs=2) as mid_pool,
            tc.tile_pool(name="w3xp", bufs=3) as w3x_pool,
            tc.tile_pool(name="outp", bufs=4) as out_pool,
            tc.tile_pool(name="small", bufs=3) as small_pool,
            tc.tile_pool(name="ps_t", bufs=2, space=bass.MemorySpace.PSUM) as ps_t,
            tc.tile_pool(name="ps_d", bufs=2, space=bass.MemorySpace.PSUM) as ps_d,
            tc.tile_pool(name="ps_s", bufs=2, space=bass.MemorySpace.PSUM) as ps_s,
        ):
            wpk = consts.tile([P, N_CHUNKS * 4], f32)
            nc.sync.dma_start(wpk[:], wpk_d[:])
            w3bc = consts.tile([P, D], f32)
            nc.sync.dma_start(w3bc[:], w3bc_d[:])
            ident = consts.tile([P, P], f32)
            nc.sync.dma_start(ident[:], ident_d[:])
            if not b_zero:
                cvec = consts.tile([P, 4], f32)
                nc.sync.dma_start(cvec[:], cvec_d[:])
                b3bc = consts.tile([P, D], f32)
                nc.sync.dma_start(b3bc[:], b3bc_d[:])

            # Prologue: absorb each const-DMA completion into one engine
            # observation up front. The LDWEIGHTS side of a matmul encodes
            # only one sync wait, so steady-state matmuls must not need two
            # fresh semaphore waits (walrus: "Too many sync wait commands").
            prol0 = ps_t.tile([P, P], f32, name="prol0", tag="xT_ps")
            nc.tensor.transpose(prol0[:], ident[:], ident[:])
            prol1 = ps_d.tile([4, P], f32, name="prol1", tag="dots_ps")
            nc.tensor.matmul(prol1[:], wpk[:, 0:4], ident[:], start=True, stop=True)
            prolv = small_pool.tile([P, 1], f32, name="prolv")
            nc.vector.tensor_copy(prolv[:], w3bc[:, 0:1])
            if not b_zero:
                prolc = small_pool.tile([P, 1], f32, name="prolc")
                nc.vector.tensor_copy(prolc[:], cvec[:, 0:1])
                prolb = small_pool.tile([P, 1], f32, name="prolb")
                nc.vector.tensor_copy(prolb[:], b3bc[:, 0:1])

            for g in range(N_PAIRS):
                xts = []
                xT_pair = mid_pool.tile([P, 2 * D], f32, name="xT_pair")
                for s in range(2):
                    t = 2 * g + s
                    xt = xin_pool.tile([P, D], f32, name="xt")
                    nc.sync.dma_start(xt[:], x_d[t * P:(t + 1) * P, :])
                    xts.append(xt)

                    # x^T per chunk: xT[p, c*128+r] = x[r, c*128+p]
                    xT_ps = ps_t.tile([P, D], f32, name="xT_ps")
                    for c in range(N_CHUNKS):
                        nc.tensor.transpose(
                            xT_ps[:, c * P:(c + 1) * P],
                            xt[:, c * P:(c + 1) * P],
                            ident[:],
                        )
                    nc.scalar.copy(xT_pair[:, s * D:(s + 1) * D], xT_ps[:])

                # dots[i, s*128+r] = [X, W0, W1, W2] for row r of subtile s,
                # N=256 moving operand covering both subtiles per chunk
                xT_v = xT_pair.rearrange("p (s d) -> p s d", s=2)
                dots_ps = ps_d.tile([4, 2 * P], f32, name="dots_ps")
                for c in range(N_CHUNKS):
                    nc.tensor.matmul(
                        dots_ps[:],
                        wpk[:, c * 4:(c + 1) * 4],
                        xT_v[:, :, c * P:(c + 1) * P],
                        start=(c == 0),
                        stop=(c == N_CHUNKS - 1),
                    )
                dots = small_pool.tile([4, 2 * P], f32, name="dots")
                nc.scalar.copy(dots[:], dots_ps[:])

                # row-major dots: dT[r, s*4+i]
                dT_ps = ps_s.tile([P, 8], f32, name="dT_ps")
                for s in range(2):
                    nc.tensor.transpose(
                        dT_ps[:, s * 4:(s + 1) * 4],
                        dots[:, s * P:(s + 1) * P],
                        ident[0:4, 0:4],
                    )
                dT = small_pool.tile([P, 8], f32, name="dT")
                nc.vector.tensor_copy(dT[:], dT_ps[:])

                # scalar recursion S_{i+1} = S_i * W_i + (X + c_i), on ACT:
                # out = Identity(in * scale + bias), scale/bias per-partition
                svec = small_pool.tile([P, 8], f32, name="svec")
                if not b_zero:
                    avec = small_pool.tile([P, 8], f32, name="avec")
                for s in range(2):
                    X = dT[:, s * 4:s * 4 + 1]
                    if b_zero:
                        addends = [X, X, X]
                    else:
                        for i in range(3):
                            nc.scalar.activation(
                                avec[:, s * 4 + i:s * 4 + i + 1],
                                X,
                                Act.Identity,
                                bias=cvec[:, i:i + 1],
                                scale=1.0,
                            )
                        addends = [
                            avec[:, s * 4 + i:s * 4 + i + 1] for i in range(3)
                        ]
                    s_prev = X
                    for i in range(3):
                        nc.scalar.activation(
                            svec[:, s * 4 + i:s * 4 + i + 1],
                            dT[:, s * 4 + i + 1:s * 4 + i + 2],
                            Act.Identity,
                            bias=addends[i],
                            scale=s_prev,
                        )
                        s_prev = svec[:, s * 4 + i:s * 4 + i + 1]

                # out = S3 * (w3 ⊙ x) + x (+ b3)
                for s in range(2):
                    t = 2 * g + s
                    xt = xts[s]
                    S3 = svec[:, s * 4 + 2:s * 4 + 3]
                    w3x = w3x_pool.tile([P, D], f32, name="w3x")
                    nc.vector.tensor_mul(w3x[:], xt[:], w3bc[:])
                    out_sb = out_pool.tile([P, D], f32, name="out_sb")
                    nc.vector.scalar_tensor_tensor(
                        out_sb[:], w3x[:], S3, xt[:], Alu.mult, Alu.add
                    )
                    if not b_zero:
                        out2 = out_pool.tile([P, D], f32, name="out2")
                        nc.vector.tensor_add(out2[:], out_sb[:], b3bc[:])
                        out_sb = out2
                    nc.sync.dma_start(out_d[t * P:(t + 1) * P, :], out_sb[:])
    nc.compile()
    return nc


def _get_nc(b_zero: bool):
    if b_zero not in _built:
        _built[b_zero] = _build_nc(b_zero)
    return _built[b_zero]


def _host_prep(w, b, b_zero):
    # Wpk[p, c*4+i] packs column i of [ones, w0, w1, w2] for D-chunk c
    M = np.empty((D, 4), dtype=np.float32)
    M[:, 0] = 1.0
    M[:, 1] = w[0]
    M[:, 2] = w[1]
    M[:, 3] = w[2]
    wpk = np.ascontiguousarray(
        M.reshape(N_CHUNKS, P, 4).transpose(1, 0, 2).reshape(P, N_CHUNKS * 4)
    )
    w3bc = np.ascontiguousarray(np.broadcast_to(w[3], (P, D)).astype(np.float32))
    ident = np.eye(P, dtype=np.float32)
    extras = {}
    if not b_zero:
        c = b.sum(axis=1).astype(np.float32)  # (L,)
        extras["cvec"] = np.ascontiguousarray(np.broadcast_to(c, (P, L)))
        extras["b3bc"] = np.ascontiguousarray(
            np.broadcast_to(b[3], (P, D)).astype(np.float32)
        )
    return wpk, w3bc, ident, extras


def kernel(inputs, w, b):
    from concourse.bass_utils import run_bass_kernel_spmd

    x = np.ascontiguousarray(np.asarray(inputs, dtype=np.float32).reshape(B, D))
    w = np.asarray(w, dtype=np.float32)
    b = np.asarray(b, dtype=np.float32)
    b_zero = not b.any()

    nc = _get_nc(b_zero)
    wpk, w3bc, ident, extras = _host_prep(w, b, b_zero)

    in_maps = []
    for i in range(N_CORES):
        m = {
            "x": x[i * RPC:(i + 1) * RPC],
            "wpk": wpk,
            "w3bc": w3bc,
            "ident": ident,
        }
        m.update(extras)
        in_maps.append(m)

    trace = bool(int(os.environ.get("KERNEL_TRACE", "0")))
    kwargs = {}
    if trace:
        kwargs = {"trace": True, "trace_cores": [0]}
    res = run_bass_kernel_spmd(nc, in_maps, core_ids=list(range(N_CORES)), **kwargs)
    if trace:
        kernel.last_results = res
    return np.concatenate([r["out"] for r in res.results], axis=0)


# revision 15
# speedup vs baseline: 1.2879x; 1.2879x over previous
"""Trainium2 Bass kernel for the CrossLayer problem.

Math: reference computes, per row x (length D), with cur_0 = x:
    cur_{i+1} = sum(cur_i) * (w_i ⊙ x) + b_i + x        (i = 0..L-1)
Only the scalar s_i = sum(cur_i) couples elements, so with
    X   = sum(x)                  (per row)
    W_i = x · w_i                 (per row, i = 0..L-2)
    c_i = sum(b_i)
the recursion collapses to scalars:
    S_0 = X;  S_{i+1} = S_i * W_i + c_i + X
and the output is a single elementwise pass:
    out = S_{L-1} * (w_{L-1} ⊙ x) + b_{L-1} + x

Kernel layout (per core, pure data parallel over batch):
  - rows on partitions, 16 tiles of (128, 1024) f32, processed in pairs
  - PE transposes each tile chunk (128x128); ACT copies PSUM→SBUF into a
    paired buffer, then the tensor engine computes [X, W0, W1, W2] =
    Wpk^T @ x^T with N=256 moving (both tiles of the pair at once)
  - small PE transpose puts the dots row-major; ACT runs the scalar
    recursion (activation Identity with per-partition scale/bias)
  - final output on DVE: tensor_mul (w3 ⊙ x) + fused scalar_tensor_tensor
    (S3 * w3x + x); the general-b path adds b3 with one more pass
"""

import os
import numpy as np

B, D, L = 16384, 1024, 4
N_CORES = 8
RPC = B // N_CORES          # rows per core
P = 128                     # partitions
N_TILES = RPC // P          # 16
N_PAIRS = N_TILES // 2      # 8
N_CHUNKS = D // P           # 8

_built = {}


def _build_nc(b_zero: bool):
    import concourse.bass as bass
    import concourse.bacc as bacc
    import concourse.mybir as mybir
    from concourse import tile

    f32 = mybir.dt.float32
    Alu = mybir.AluOpType
    Act = mybir.ActivationFunctionType

    # Bacc (not raw Bass): its compile() legalizes semaphore waits — TRN2
    # matmuls encode at most one sync wait (walrus S3_LW struct).
    nc = bacc.Bacc(
        "TRN2", target_bir_lowering=False, debug=False, num_devices=N_CORES
    )
    x_d = nc.dram_tensor("x", [RPC, D], f32, kind="ExternalInput")
    wpk_d = nc.dram_tensor("wpk", [P, N_CHUNKS * 4], f32, kind="ExternalInput")
    w3bc_d = nc.dram_tensor("w3bc", [P, D], f32, kind="ExternalInput")
    ident_d = nc.dram_tensor("ident", [P, P], f32, kind="ExternalInput")
    if not b_zero:
        cvec_d = nc.dram_tensor("cvec", [P, 4], f32, kind="ExternalInput")
        b3bc_d = nc.dram_tensor("b3bc", [P, D], f32, kind="ExternalInput")
    out_d = nc.dram_tensor("out", [RPC, D], f32, kind="ExternalOutput")

    with tile.TileContext(nc) as tc:
        with (
            tc.tile_pool(name="consts", bufs=1) as consts,
            tc.tile_pool(name="xin", bufs=N_TILES) as xin_pool,
            tc.tile_pool(name="mid", bufs=4) as mid_pool,
            tc.tile_pool(name="w3xp", bufs=4) as w3x_pool,
            tc.tile_pool(name="outp", bufs=5) as out_pool,
            tc.tile_pool(name="small", bufs=3) as small_pool,
            tc.tile_pool(name="ps_t", bufs=4, space=bass.MemorySpace.PSUM) as ps_t,
            tc.tile_pool(name="ps_d", bufs=2, space=bass.MemorySpace.PSUM) as ps_d,
            tc.tile_pool(name="ps_s", bufs=2, space=bass.MemorySpace.PSUM) as ps_s,
        ):
            wpk = consts.tile([P, N_CHUNKS * 4], f32)
            nc.sync.dma_start(wpk[:], wpk_d[:])
            w3bc = consts.tile([P, D], f32)
            nc.sync.dma_start(w3bc[:], w3bc_d[:])
            ident = consts.tile([P, P], f32)
            nc.sync.dma_start(ident[:], ident_d[:])
            if not b_zero:
                cvec = consts.tile([P, 4], f32)
                nc.sync.dma_start(cvec[:], cvec_d[:])
                b3bc = consts.tile([P, D], f32)
                nc.sync.dma_start(b3bc[:], b3bc_d[:])

            # Prologue: absorb each const-DMA completion into one engine
            # observation up front. The LDWEIGHTS side of a matmul encodes
            # only one sync wait, so steady-state matmuls must not need two
            # fresh semaphore waits (walrus: "Too many sync wait commands").
            prol0 = ps_t.tile([P, D // 2], f32, name="prol0", tag="xT_ps")
            nc.tensor.transpose(prol0[0:P, 0:P], ident[:], ident[:])
            prol1 = ps_d.tile([4, P], f32, name="prol1", tag="dots_ps")
            nc.tensor.matmul(prol1[:], wpk[:, 0:4], ident[:], start=True, stop=True)
            prolv = small_pool.tile([P, 1], f32, name="prolv")
            nc.vector.tensor_copy(prolv[:], w3bc[:, 0:1])
            if not b_zero:
                prolc = small_pool.tile([P, 1], f32, name="prolc")
                nc.vector.tensor_copy(prolc[:], cvec[:, 0:1])
                prolb = small_pool.tile([P, 1], f32, name="prolb")
                nc.vector.tensor_copy(prolb[:], b3bc[:, 0:1])

            for t in range(N_TILES):
                xt = xin_pool.tile([P, D], f32, name="xt")
                nc.sync.dma_start(xt[:], x_d[t * P:(t + 1) * P, :])

                # w3 ⊙ x only needs xt — emit early so the DVE tail is short
                w3x = w3x_pool.tile([P, D], f32, name="w3x")
                nc.vector.tensor_mul(w3x[:], xt[:], w3bc[:])

                # x^T per chunk in two half-tiles (1 PSUM bank each):
                # xT[p, c*128+r] = x[r, c*128+p]
                xT_halves = []
                for h in range(2):
                    xT_ps = ps_t.tile([P, D // 2], f32, name="xT_ps")
                    for cc in range(N_CHUNKS // 2):
                        c = h * (N_CHUNKS // 2) + cc
                        nc.tensor.transpose(
                            xT_ps[:, cc * P:(cc + 1) * P],
                            xt[:, c * P:(c + 1) * P],
                            ident[:],
                        )
                    xT_h = mid_pool.tile([P, D // 2], f32, name="xT_h")
                    nc.scalar.copy(xT_h[:], xT_ps[:])
                    xT_halves.append(xT_h)

                # dots[i, r] = [X, W0, W1, W2][r], accumulated over chunks
                dots_ps = ps_d.tile([4, P], f32, name="dots_ps")
                for c in range(N_CHUNKS):
                    h, cc = divmod(c, N_CHUNKS // 2)
                    nc.tensor.matmul(
                        dots_ps[:],
                        wpk[:, c * 4:(c + 1) * 4],
                        xT_halves[h][:, cc * P:(cc + 1) * P],
                        start=(c == 0),
                        stop=(c == N_CHUNKS - 1),
                    )
                dots = small_pool.tile([4, P], f32, name="dots")
                nc.scalar.copy(dots[:], dots_ps[:])

                # back to row-major: dT[r, i]
                dT_ps = ps_s.tile([P, 4], f32, name="dT_ps")
                nc.tensor.transpose(dT_ps[:], dots[:], ident[0:4, 0:4])
                dT = small_pool.tile([P, 4], f32, name="dT")
                nc.scalar.copy(dT[:], dT_ps[:])

                # scalar recursion S_{i+1} = S_i * W_i + (X + c_i)
                svec = small_pool.tile([P, 4], f32, name="svec")
                X = dT[:, 0:1]
                if b_zero:
                    addends = [X, X, X]
                else:
                    avec = small_pool.tile([P, 4], f32, name="avec")
                    for i in range(3):
                        nc.vector.tensor_scalar_add(
                            avec[:, i:i + 1], X, cvec[:, i:i + 1]
                        )
                    addends = [avec[:, 0:1], avec[:, 1:2], avec[:, 2:3]]
                s_prev = X
                for i in range(3):
                    nc.vector.tensor_scalar(
                        svec[:, i:i + 1],
                        s_prev,
                        dT[:, i + 1:i + 2],
                        addends[i],
                        Alu.mult,
                        Alu.add,
                    )
                    s_prev = svec[:, i:i + 1]
                S3 = svec[:, 2:3]

                # out = S3 * (w3 ⊙ x) + x (+ b3)
                out_sb = out_pool.tile([P, D], f32, name="out_sb")
                nc.vector.scalar_tensor_tensor(
                    out_sb[:], w3x[:], S3, xt[:], Alu.mult, Alu.add
                )
                if not b_zero:
                    out2 = out_pool.tile([P, D], f32, name="out2")
                    nc.vector.tensor_add(out2[:], out_sb[:], b3bc[:])
                    out_sb = out2
                nc.sync.dma_start(out_d[t * P:(t + 1) * P, :], out_sb[:])
    nc.compile()
    return nc


def _get_nc(b_zero: bool):
    if b_zero not in _built:
        _built[b_zero] = _build_nc(b_zero)
    return _built[b_zero]


def _host_prep(w, b, b_zero):
    # Wpk[p, c*4+i] packs column i of [ones, w0, w1, w2] for D-chunk c
    M = np.empty((D, 4), dtype=np.float32)
    M[:, 0] = 1.0
    M[:, 1] = w[0]
    M[:, 2] = w[1]
    M[:, 3] = w[2]
    wpk = np.ascontiguousarray(
        M.reshape(N_CHUNKS, P, 4).transpose(1, 0, 2).reshape(P, N_CHUNKS * 4)
    )
    w3bc = np.ascontiguousarray(np.broadcast_to(w[3], (P, D)).astype(np.float32))
    ident = np.eye(P, dtype=np.float32)
    extras = {}
    if not b_zero:
        c = b.sum(axis=1).astype(np.float32)  # (L,)
        extras["cvec"] = np.ascontiguousarray(np.broadcast_to(c, (P, L)))
        extras["b3bc"] = np.ascontiguousarray(
            np.broadcast_to(b[3], (P, D)).astype(np.float32)
        )
    return wpk, w3bc, ident, extras


def kernel(inputs, w, b):
    from concourse.bass_utils import run_bass_kernel_spmd

    x = np.ascontiguousarray(np.asarray(inputs, dtype=np.float32).reshape(B, D))
    w = np.asarray(w, dtype=np.float32)
    b = np.asarray(b, dtype=np.float32)
    b_zero = not b.any()

    nc = _get_nc(b_zero)
    wpk, w3bc, ident, extras = _host_prep(w, b, b_zero)

    in_maps = []
    for i in range(N_CORES):
        m = {
            "x": x[i * RPC:(i + 1) * RPC],
            "wpk": wpk,
            "w3bc": w3bc,
            "ident": ident,
        }
        m.update(extras)
        in_maps.append(m)

    trace = bool(int(os.environ.get("KERNEL_TRACE", "0")))
    kwargs = {}
    if trace:
        kwargs = {"trace": True, "trace_cores": [0]}
    res = run_bass_kernel_spmd(nc, in_maps, core_ids=list(range(N_CORES)), **kwargs)
    if trace:
        kernel.last_results = res
    return np.concatenate([r["out"] for r in res.results], axis=0)
